# revision 5
# baseline (speedup 1.0000x reference)
"""Trainium2 Bass kernel for CumsumAttention.

Full-input contract: kernel(**inputs) takes the complete (unsharded) inputs
and returns the full [B, T, C] float32 output. Internally the work is
data-parallel over the batch dimension across 8 NeuronCores (2 batches per
core); each core runs the same Bass/Tile program on its own batch shard.

Math (per batch, reference semantics):
  k = x @ Wk.T + bk ; q = x @ Wq.T + bq ; v = x @ Wv.T + bv
  angle[t] = sum_{s>=t} ang_emb[idx[s]]        (reverse cumsum over T)
  rot/inv_rot: per channel-pair rotation by angle
  wei  = softplus((rot(k) @ rot(q).T) / sqrt(C)) masked causally (s <= t)
  out  = inv_rot(wei @ rot(v)) @ Wp.T + bp

Device implementation notes:
  - Channel pairs (2i, 2i+1) are made contiguous by a host-side permutation
    baked into the weights, so the rotation operates on [0:C/2] / [C/2:C]
    slabs (elementwise, fp16 on DVE).
  - All matmul operands are fp16 (fp32 PSUM accumulation).  The reverse
    cumsum is a lower-triangular-ones matmul.  cos/sin come from the ACT
    Sin spline after an add_range_wrap range reduction; softplus is
    exp -> ln(1+x) (both in the natural_log_exp table set).
  - Attention is computed as wei^T (shape [s, t]) so both attention matmuls
    and the final projection consume operands in natural layouts; only
    cos/sin need an on-device transpose (PE transpose mode).
"""

import sys
import types
from contextlib import ExitStack

import numpy as np

if "/opt/trn_rl_repo" not in sys.path:
    sys.path.insert(0, "/opt/trn_rl_repo")

B, T, C = 16, 1024, 1024
D = C // 2
NCORES = 8
BPC = B // NCORES          # batches per core
P = 128                    # partitions
NT = T // P                # t tiles
NCH = C // P               # c tiles
ND = D // P                # d tiles (channel pairs)
H = 512                    # matmul free-dim block
PI = float(np.pi)
SCALE = float(C ** -0.5)

_CACHE = {}


def _install_profile_hook():
    """Register the axon NTFF profile hook if the image's antenv lacks it.

    Harmless when tracing is never requested; lets test harnesses pass
    trace=True to run_bass_kernel_spmd for HW timing.
    """
    try:
        import antenv
        from trn_agent_boot.trn_boot import _ntff_profile_via_ctypes
    except Exception:
        return
    if "antenv.axon_hooks" in sys.modules:
        return
    try:
        hook = _ntff_profile_via_ctypes("/opt/axon/libaxon_pjrt.so")
    except Exception:
        return
    mod = types.ModuleType("antenv.axon_hooks")
    mod.get_axon_ntff_profile_hook = lambda: hook
    mod.set_axon_ntff_profile_hook = lambda h: None
    sys.modules["antenv.axon_hooks"] = mod
    antenv.axon_hooks = mod


def _build(has_bias: bool):
    import concourse.bass as bass  # noqa: F401
    import concourse.mybir as mybir
    import concourse.tile as tile
    from concourse import bacc
    from concourse.masks import make_identity

    dt = mybir.dt
    AF = mybir.ActivationFunctionType
    OP = mybir.AluOpType
    f16 = dt.float16
    f32 = dt.float32

    nc = bacc.Bacc("TRN2", target_bir_lowering=False, debug=False,
                   num_devices=NCORES)

    xT_d = nc.dram_tensor("xT", [BPC, NCH, P, T], f16, kind="ExternalInput").ap()
    ang_d = nc.dram_tensor("ang", [BPC, NT, P, D], f16, kind="ExternalInput").ap()
    wk_d = nc.dram_tensor("wkT", [NCH, P, C], f16, kind="ExternalInput").ap()
    wq_d = nc.dram_tensor("wqT", [NCH, P, C], f16, kind="ExternalInput").ap()
    wv_d = nc.dram_tensor("wvT", [NCH, P, C], f16, kind="ExternalInput").ap()
    wp_d = nc.dram_tensor("wpT", [NCH, P, C], f16, kind="ExternalInput").ap()
    tril_d = nc.dram_tensor("tril", [P, P], f16, kind="ExternalInput").ap()
    ones_d = nc.dram_tensor("onesb", [P, P], f16, kind="ExternalInput").ap()
    triu_d = nc.dram_tensor("triu", [P, P], f16, kind="ExternalInput").ap()
    if has_bias:
        bias_d = nc.dram_tensor("biases", [4, 1, C], f16, kind="ExternalInput").ap()
    y_d = nc.dram_tensor("y", [BPC, T, C], f32, kind="ExternalOutput").ap()

    with tile.TileContext(nc) as tc, ExitStack() as ctx:
        const = ctx.enter_context(tc.tile_pool(name="const", bufs=1))
        w_pool = ctx.enter_context(tc.tile_pool(name="wpool", bufs=2))
        xo_pool = ctx.enter_context(tc.tile_pool(name="xopool", bufs=2))
        a_pool = ctx.enter_context(tc.tile_pool(name="apool", bufs=1))
        st_pool = ctx.enter_context(tc.tile_pool(name="stpool", bufs=1))
        ct_pool = ctx.enter_context(tc.tile_pool(name="ctpool", bufs=1))
        sc_pool = ctx.enter_context(tc.tile_pool(name="scpool", bufs=1))
        cc_pool = ctx.enter_context(tc.tile_pool(name="ccpool", bufs=1))
        k_pool = ctx.enter_context(tc.tile_pool(name="kpool", bufs=1))
        q_pool = ctx.enter_context(tc.tile_pool(name="qpool", bufs=1))
        v_pool = ctx.enter_context(tc.tile_pool(name="vpool", bufs=1))
        spw_pool = ctx.enter_context(tc.tile_pool(name="spwpool", bufs=1))
        m_pool = ctx.enter_context(tc.tile_pool(name="mpool", bufs=2))
        sp_pool = ctx.enter_context(tc.tile_pool(name="sppool", bufs=3))
        y_pool = ctx.enter_context(tc.tile_pool(name="ypool", bufs=2))
        pmm = ctx.enter_context(tc.tile_pool(name="pmm", bufs=6, space="PSUM"))
        ptr = ctx.enter_context(tc.tile_pool(name="ptr", bufs=2, space="PSUM"))

        ident = const.tile([P, P], f16)
        make_identity(nc, ident[:])
        tril = const.tile([P, P], f16)
        onesb = const.tile([P, P], f16)
        triu = const.tile([P, P], f16)
        nc.sync.dma_start(tril[:], tril_d[:])
        nc.sync.dma_start(onesb[:], ones_d[:])
        nc.sync.dma_start(triu[:], triu_d[:])
        if has_bias:
            ones_row = const.tile([1, H], f16)
            nc.gpsimd.memset(ones_row[:], 1.0)
            brows = const.tile([1, 4, C], f16)
            for i in range(4):
                nc.sync.dma_start(brows[:, i], bias_d[i])

        for b in range(BPC):
            # ---------------- input DMA ----------------
            # a tiles are loaded high-to-low: the reverse cumsum for t-tile
            # ti consumes s-tiles ti..NT-1, so ti=NT-1 is ready first.
            a_all = a_pool.tile([P, NT, D], f16, tag="a")
            for ti in reversed(range(NT)):
                nc.sync.dma_start(a_all[:, ti], ang_d[b, ti])
            x_all = xo_pool.tile([P, NCH, T], f16, tag="xo")
            for ci in range(NCH):
                nc.sync.dma_start(x_all[:, ci], xT_d[b, ci])

            # ---------------- phase A: angle cumsum + cos/sin ----------------
            sin_t = st_pool.tile([P, NT, D], f16, tag="sin_t")
            cos_t = ct_pool.tile([P, NT, D], f16, tag="cos_t")
            for ti in reversed(range(NT)):
                ps = pmm.tile([P, D], f32, tag="pmm")
                for sj in range(NT - 1, ti - 1, -1):
                    nc.tensor.matmul(ps[:], (tril if sj == ti else onesb)[:],
                                     a_all[:, sj],
                                     start=(sj == NT - 1), stop=(sj == ti))
                u = sp_pool.tile([P, D], f32, tag="sp")
                w = sp_pool.tile([P, D], f32, tag="sp")
                nc.vector.add_range_wrap(u[:], ps[:], 0.0, PI, 2 * PI)
                nc.vector.add_range_wrap(w[:], ps[:], PI / 2, PI, 2 * PI)
                nc.scalar.activation(sin_t[:, ti], u[:], AF.Sin)
                nc.scalar.activation(cos_t[:, ti], w[:], AF.Sin)

            # transpose cos/sin to channel-major layout
            sin_c = sc_pool.tile([P, ND, T], f16, tag="sin_c")
            cos_c = cc_pool.tile([P, ND, T], f16, tag="cos_c")
            for src, dst in ((cos_t, cos_c), (sin_t, sin_c)):
                for dj in range(ND):
                    for th in range(2):
                        pst = ptr.tile([P, H], f16, tag="ptr")
                        for tq in range(4):
                            ti = th * 4 + tq
                            nc.tensor.transpose(pst[:, tq * P:(tq + 1) * P],
                                                src[:, ti, dj * P:(dj + 1) * P],
                                                ident[:])
                        nc.vector.tensor_copy(dst[:, dj, th * H:(th + 1) * H], pst[:])

            # ---------------- phase B: projections + rotation ----------------
            # v = x @ Wv'.T in [t, c] layout
            wv = w_pool.tile([P, NCH, C], f16, tag="w")
            for ci in range(NCH):
                nc.sync.dma_start(wv[:, ci], wv_d[ci])
            v_all = v_pool.tile([P, NT, C], f16, tag="v")
            for ti in range(NT):
                for ch in range(2):
                    ps = pmm.tile([P, H], f32, tag="pmm")
                    for ci in range(NCH):
                        nc.tensor.matmul(ps[:], x_all[:, ci, ti * P:(ti + 1) * P],
                                         wv[:, ci, ch * H:(ch + 1) * H],
                                         start=(ci == 0),
                                         stop=(ci == NCH - 1 and not has_bias))
                    if has_bias:
                        nc.tensor.matmul(ps[:], ones_row[:, :P],
                                         brows[:, 2, ch * H:(ch + 1) * H],
                                         start=False, stop=True)
                    nc.scalar.activation(v_all[:, ti, ch * H:(ch + 1) * H], ps[:], AF.Copy)
            # rotate v in place (t-major): pairs are column slabs [0:D], [D:C]
            for ti in range(NT):
                z0 = v_all[:, ti, 0:D]
                z1 = v_all[:, ti, D:C]
                ma = m_pool.tile([P, T], f16, tag="ma")
                mb = m_pool.tile([P, T], f16, tag="mb")
                mc = m_pool.tile([P, T], f16, tag="mc")
                nc.vector.tensor_mul(ma[:, 0:D], z0, cos_t[:, ti])
                nc.vector.tensor_mul(mb[:, 0:D], z1, sin_t[:, ti])
                nc.vector.tensor_mul(mc[:, 0:D], z0, sin_t[:, ti])
                nc.vector.tensor_sub(z0, ma[:, 0:D], mb[:, 0:D])
                nc.vector.tensor_mul(ma[:, D:C], z1, cos_t[:, ti])
                nc.vector.tensor_add(z1, mc[:, 0:D], ma[:, D:C])

            def proj_cmajor(w_dram, out_pool, tag, bias_idx):
                w_sb = w_pool.tile([P, NCH, C], f16, tag="w")
                for ci in range(NCH):
                    nc.sync.dma_start(w_sb[:, ci], w_dram[ci])
                out_all = out_pool.tile([P, NCH, T], f16, tag=tag)
                # Pair-ordered emission: compute co=p then co=p+4, rotating the
                # pair immediately so the DVE trails the PE instead of bunching
                # all rotations at the end of the projection.
                for p in range(ND):
                    for co in (p, p + 4):
                        for th in range(2):
                            ps = pmm.tile([P, H], f32, tag="pmm")
                            for ci in range(NCH):
                                nc.tensor.matmul(ps[:], w_sb[:, ci, co * P:(co + 1) * P],
                                                 x_all[:, ci, th * H:(th + 1) * H],
                                                 start=(ci == 0),
                                                 stop=(ci == NCH - 1 and not has_bias))
                            if has_bias:
                                nc.tensor.matmul(ps[:], brows[:, bias_idx, co * P:(co + 1) * P],
                                                 ones_row[:], start=False, stop=True)
                            nc.scalar.activation(out_all[:, co, th * H:(th + 1) * H],
                                                 ps[:], AF.Copy)
                    z0 = out_all[:, p, :]
                    z1 = out_all[:, p + 4, :]
                    cs = cos_c[:, p, :]
                    sn = sin_c[:, p, :]
                    ma = m_pool.tile([P, T], f16, tag="ma")
                    mb = m_pool.tile([P, T], f16, tag="mb")
                    mc = m_pool.tile([P, T], f16, tag="mc")
                    nc.vector.tensor_mul(ma[:], z0, cs)
                    nc.vector.tensor_mul(mb[:], z1, sn)
                    nc.vector.tensor_mul(mc[:], z0, sn)
                    nc.vector.tensor_sub(z0, ma[:], mb[:])
                    nc.vector.tensor_mul(ma[:], z1, cs)
                    nc.vector.tensor_add(z1, mc[:], ma[:])
                return out_all

            k_all = proj_cmajor(wk_d, k_pool, "k", 0)
            q_all = proj_cmajor(wq_d, q_pool, "q", 1)

            # ---------------- phase C: wei^T = softplus(q.k / sqrt(C)) -------
            spw = spw_pool.tile([P, NT, T], f16, tag="spw")
            for th in range(2):
                smax = 4 * th + 3
                for si in range(smax + 1):
                    ps = pmm.tile([P, H], f32, tag="pmm")
                    for ci in range(NCH):
                        nc.tensor.matmul(ps[:], q_all[:, ci, si * P:(si + 1) * P],
                                         k_all[:, ci, th * H:(th + 1) * H],
                                         start=(ci == 0), stop=(ci == NCH - 1))
                    e = sp_pool.tile([P, H], f32, tag="sp")
                    nc.scalar.activation(e[:], ps[:], AF.Exp, scale=SCALE)
                    nc.scalar.activation(spw[:, si, th * H:(th + 1) * H], e[:],
                                         AF.Ln, bias=1.0)
                    if 4 * th <= si <= 4 * th + 3:
                        off = si * P - th * H
                        if off > 0:
                            nc.vector.memset(spw[:, si, th * H:th * H + off], 0.0)
                        dcol = th * H + off
                        nc.vector.tensor_mul(spw[:, si, dcol:dcol + P],
                                             spw[:, si, dcol:dcol + P], triu[:])

            # ---------------- phase D: out^T = v.T @ wei^T, inverse-rotated --
            ro = xo_pool.tile([P, NCH, T], f16, tag="xo")
            for pp in range(ND):
                for th in range(2):
                    smax = 4 * th + 3
                    ps0 = pmm.tile([P, H], f32, tag="pmm")
                    ps1 = pmm.tile([P, H], f32, tag="pmm")
                    for sj in range(smax + 1):
                        nc.tensor.matmul(ps0[:], v_all[:, sj, pp * P:(pp + 1) * P],
                                         spw[:, sj, th * H:(th + 1) * H],
                                         start=(sj == 0), stop=(sj == smax))
                    for sj in range(smax + 1):
                        nc.tensor.matmul(ps1[:], v_all[:, sj, (pp + 4) * P:(pp + 5) * P],
                                         spw[:, sj, th * H:(th + 1) * H],
                                         start=(sj == 0), stop=(sj == smax))
                    cs = cos_c[:, pp, th * H:(th + 1) * H]
                    sn = sin_c[:, pp, th * H:(th + 1) * H]
                    # Evacuate PSUM via ACT first so the DVE rotation runs in
                    # 2x mode on all-fp16 SBUF operands (PSUM-sourced
                    # tensor_tensor is 1x and stalls the PE on PSUM slots).
                    oz = m_pool.tile([P, T], f16, tag="oz")
                    nc.scalar.activation(oz[:, 0:H], ps0[:], AF.Copy)
                    nc.scalar.activation(oz[:, H:T], ps1[:], AF.Copy)
                    ma = m_pool.tile([P, T], f16, tag="ma")
                    mb = m_pool.tile([P, T], f16, tag="mb")
                    # inv_rot: o0 = cos*z0 + sin*z1 ; o1 = cos*z1 - sin*z0
                    nc.vector.tensor_mul(ma[:, 0:H], oz[:, 0:H], cs)
                    nc.vector.tensor_mul(mb[:, 0:H], oz[:, H:T], sn)
                    nc.vector.tensor_add(ro[:, pp, th * H:(th + 1) * H],
                                         ma[:, 0:H], mb[:, 0:H])
                    nc.vector.tensor_mul(ma[:, H:T], oz[:, 0:H], sn)
                    nc.vector.tensor_mul(mb[:, H:T], oz[:, H:T], cs)
                    nc.vector.tensor_sub(ro[:, pp + 4, th * H:(th + 1) * H],
                                         mb[:, H:T], ma[:, H:T])

            # ---------------- phase E: y = ro.T @ Wp'.T ----------------------
            wp = w_pool.tile([P, NCH, C], f16, tag="w")
            for ci in range(NCH):
                nc.sync.dma_start(wp[:, ci], wp_d[ci])
            for ti in range(NT):
                for ch in range(2):
                    ps = pmm.tile([P, H], f32, tag="pmm")
                    for ci in range(NCH):
                        nc.tensor.matmul(ps[:], ro[:, ci, ti * P:(ti + 1) * P],
                                         wp[:, ci, ch * H:(ch + 1) * H],
                                         start=(ci == 0),
                                         stop=(ci == NCH - 1 and not has_bias))
                    if has_bias:
                        nc.tensor.matmul(ps[:], ones_row[:, :P],
                                         brows[:, 3, ch * H:(ch + 1) * H],
                                         start=False, stop=True)
                    yt = y_pool.tile([P, H], f32, tag="y")
                    nc.scalar.activation(yt[:], ps[:], AF.Copy)
                    nc.sync.dma_start(y_d[b, ti * P:(ti + 1) * P, ch * H:(ch + 1) * H],
                                      yt[:])

    nc.compile()
    return nc


def _get_program(has_bias: bool):
    key = ("prog", has_bias)
    if key not in _CACHE:
        _CACHE[key] = _build(has_bias)
    return _CACHE[key]


def _prep_host(x, idx, Wk, Wq, Wv, Wp, ang_emb, biases):
    perm = np.concatenate([np.arange(0, C, 2), np.arange(1, C, 2)])
    xT = np.ascontiguousarray(np.transpose(np.asarray(x, np.float32), (0, 2, 1)))
    xT16 = xT.astype(np.float16).reshape(NCORES, BPC, NCH, P, T)
    idx = np.asarray(idx).astype(np.int64)
    ang = np.asarray(ang_emb, np.float32)[idx]          # [B, T, D]
    ang16 = ang.astype(np.float16).reshape(NCORES, BPC, NT, P, D)

    def wtile(m):
        return np.ascontiguousarray(m).astype(np.float16).reshape(NCH, P, C)

    wkT = wtile(np.asarray(Wk, np.float32)[perm].T)
    wqT = wtile(np.asarray(Wq, np.float32)[perm].T)
    wvT = wtile(np.asarray(Wv, np.float32)[perm].T)
    wpT = wtile(np.asarray(Wp, np.float32)[:, perm].T)

    tril = np.tril(np.ones((P, P), np.float16))
    onesb = np.ones((P, P), np.float16)
    triu = np.triu(np.ones((P, P), np.float16))

    consts = dict(wkT=wkT, wqT=wqT, wvT=wvT, wpT=wpT,
                  tril=tril, onesb=onesb, triu=triu)
    bk, bq, bv, bp = (np.asarray(b_, np.float32) for b_ in biases)
    has_bias = any(np.any(b_ != 0) for b_ in (bk, bq, bv, bp))
    if has_bias:
        brows = np.stack([bk[perm], bq[perm], bv[perm], bp]).astype(np.float16)
        consts["biases"] = brows.reshape(4, 1, C)
    return xT16, ang16, consts, has_bias


def run_on_device(x, idx, Wk, Wq, Wv, Wp, ang_emb, biases, trace=False):
    _install_profile_hook()
    import concourse.bass_utils as bass_utils
    bass_utils.upload_artifacts = lambda tmpdir: "local://" + tmpdir
    from concourse.bass_utils import run_bass_kernel_spmd

    xT16, ang16, consts, has_bias = _prep_host(x, idx, Wk, Wq, Wv, Wp,
                                               ang_emb, biases)
    nc = _get_program(has_bias)
    in_maps = []
    for c in range(NCORES):
        m = {"xT": xT16[c], "ang": ang16[c]}
        m.update(consts)
        in_maps.append(m)
    res = run_bass_kernel_spmd(nc, in_maps, list(range(NCORES)), trace=trace)
    y = np.empty((B, T, C), np.float32)
    for c in range(NCORES):
        y[c * BPC:(c + 1) * BPC] = res.results[c]["y"]
    return y, res


def kernel(x, idx, Wk, bk, Wq, bq, Wv, bv, Wp, bp, ang_emb):
    y, _ = run_on_device(x, idx, Wk, Wq, Wv, Wp, ang_emb, (bk, bq, bv, bp))
    return y


# revision 6
# speedup vs baseline: 1.0179x; 1.0179x over previous
"""Trainium2 Bass kernel for CumsumAttention.

Full-input contract: kernel(**inputs) takes the complete (unsharded) inputs
and returns the full [B, T, C] float32 output. Internally the work is
data-parallel over the batch dimension across 8 NeuronCores (2 batches per
core); each core runs the same Bass/Tile program on its own batch shard.

Math (per batch, reference semantics):
  k = x @ Wk.T + bk ; q = x @ Wq.T + bq ; v = x @ Wv.T + bv
  angle[t] = sum_{s>=t} ang_emb[idx[s]]        (reverse cumsum over T)
  rot/inv_rot: per channel-pair rotation by angle
  wei  = softplus((rot(k) @ rot(q).T) / sqrt(C)) masked causally (s <= t)
  out  = inv_rot(wei @ rot(v)) @ Wp.T + bp

Device implementation notes:
  - Channel pairs (2i, 2i+1) are made contiguous by a host-side permutation
    baked into the weights, so the rotation operates on [0:C/2] / [C/2:C]
    slabs (elementwise, fp16 on DVE).
  - All matmul operands are fp16 (fp32 PSUM accumulation).  The reverse
    cumsum is a lower-triangular-ones matmul.  cos/sin come from the ACT
    Sin spline after an add_range_wrap range reduction; softplus is
    exp -> ln(1+x) (both in the natural_log_exp table set).
  - Attention is computed as wei^T (shape [s, t]) so both attention matmuls
    and the final projection consume operands in natural layouts; only
    cos/sin need an on-device transpose (PE transpose mode).
"""

import sys
import types
from contextlib import ExitStack

import numpy as np

if "/opt/trn_rl_repo" not in sys.path:
    sys.path.insert(0, "/opt/trn_rl_repo")

B, T, C = 16, 1024, 1024
D = C // 2
NCORES = 8
BPC = B // NCORES          # batches per core
P = 128                    # partitions
NT = T // P                # t tiles
NCH = C // P               # c tiles
ND = D // P                # d tiles (channel pairs)
H = 512                    # matmul free-dim block
PI = float(np.pi)
SCALE = float(C ** -0.5)

_CACHE = {}


def _install_profile_hook():
    """Register the axon NTFF profile hook if the image's antenv lacks it.

    Harmless when tracing is never requested; lets test harnesses pass
    trace=True to run_bass_kernel_spmd for HW timing.
    """
    try:
        import antenv
        from trn_agent_boot.trn_boot import _ntff_profile_via_ctypes
    except Exception:
        return
    if "antenv.axon_hooks" in sys.modules:
        return
    try:
        hook = _ntff_profile_via_ctypes("/opt/axon/libaxon_pjrt.so")
    except Exception:
        return
    mod = types.ModuleType("antenv.axon_hooks")
    mod.get_axon_ntff_profile_hook = lambda: hook
    mod.set_axon_ntff_profile_hook = lambda h: None
    sys.modules["antenv.axon_hooks"] = mod
    antenv.axon_hooks = mod


def _build(has_bias: bool):
    import concourse.bass as bass  # noqa: F401
    import concourse.mybir as mybir
    import concourse.tile as tile
    from concourse import bacc
    from concourse.masks import make_identity

    dt = mybir.dt
    AF = mybir.ActivationFunctionType
    OP = mybir.AluOpType
    f16 = dt.float16
    f32 = dt.float32

    nc = bacc.Bacc("TRN2", target_bir_lowering=False, debug=False,
                   num_devices=NCORES)

    xT_d = nc.dram_tensor("xT", [BPC, NCH, P, T], f16, kind="ExternalInput").ap()
    ang_d = nc.dram_tensor("ang", [BPC, NT, P, D], f16, kind="ExternalInput").ap()
    wk_d = nc.dram_tensor("wkT", [NCH, P, C], f16, kind="ExternalInput").ap()
    wq_d = nc.dram_tensor("wqT", [NCH, P, C], f16, kind="ExternalInput").ap()
    wv_d = nc.dram_tensor("wvT", [NCH, P, C], f16, kind="ExternalInput").ap()
    wp_d = nc.dram_tensor("wpT", [NCH, P, C], f16, kind="ExternalInput").ap()
    tril_d = nc.dram_tensor("tril", [P, P], f16, kind="ExternalInput").ap()
    ones_d = nc.dram_tensor("onesb", [P, P], f16, kind="ExternalInput").ap()
    triu_d = nc.dram_tensor("triu", [P, P], f16, kind="ExternalInput").ap()
    if has_bias:
        bias_d = nc.dram_tensor("biases", [4, 1, C], f16, kind="ExternalInput").ap()
    y_d = nc.dram_tensor("y", [BPC, T, C], f32, kind="ExternalOutput").ap()

    with tile.TileContext(nc) as tc, ExitStack() as ctx:
        const = ctx.enter_context(tc.tile_pool(name="const", bufs=1))
        w_pool = ctx.enter_context(tc.tile_pool(name="wpool", bufs=2))
        xo_pool = ctx.enter_context(tc.tile_pool(name="xopool", bufs=2))
        a_pool = ctx.enter_context(tc.tile_pool(name="apool", bufs=1))
        st_pool = ctx.enter_context(tc.tile_pool(name="stpool", bufs=1))
        ct_pool = ctx.enter_context(tc.tile_pool(name="ctpool", bufs=1))
        sc_pool = ctx.enter_context(tc.tile_pool(name="scpool", bufs=1))
        cc_pool = ctx.enter_context(tc.tile_pool(name="ccpool", bufs=1))
        k_pool = ctx.enter_context(tc.tile_pool(name="kpool", bufs=1))
        q_pool = ctx.enter_context(tc.tile_pool(name="qpool", bufs=1))
        v_pool = ctx.enter_context(tc.tile_pool(name="vpool", bufs=1))
        spw_pool = ctx.enter_context(tc.tile_pool(name="spwpool", bufs=1))
        m_pool = ctx.enter_context(tc.tile_pool(name="mpool", bufs=2))
        sp_pool = ctx.enter_context(tc.tile_pool(name="sppool", bufs=3))
        y_pool = ctx.enter_context(tc.tile_pool(name="ypool", bufs=2))
        pmm = ctx.enter_context(tc.tile_pool(name="pmm", bufs=6, space="PSUM"))
        ptr = ctx.enter_context(tc.tile_pool(name="ptr", bufs=2, space="PSUM"))

        ident = const.tile([P, P], f16)
        make_identity(nc, ident[:])
        tril = const.tile([P, P], f16)
        onesb = const.tile([P, P], f16)
        triu = const.tile([P, P], f16)
        nc.sync.dma_start(tril[:], tril_d[:])
        nc.sync.dma_start(onesb[:], ones_d[:])
        nc.sync.dma_start(triu[:], triu_d[:])
        if has_bias:
            ones_row = const.tile([1, H], f16)
            nc.gpsimd.memset(ones_row[:], 1.0)
            brows = const.tile([1, 4, C], f16)
            for i in range(4):
                nc.sync.dma_start(brows[:, i], bias_d[i])

        for b in range(BPC):
            # ---------------- input DMA ----------------
            # a tiles are loaded high-to-low: the reverse cumsum for t-tile
            # ti consumes s-tiles ti..NT-1, so ti=NT-1 is ready first.
            a_all = a_pool.tile([P, NT, D], f16, tag="a")
            for ti in reversed(range(NT)):
                nc.sync.dma_start(a_all[:, ti], ang_d[b, ti])
            x_all = xo_pool.tile([P, NCH, T], f16, tag="xo")
            for ci in range(NCH):
                nc.sync.dma_start(x_all[:, ci], xT_d[b, ci])

            # ---------------- phase A: angle cumsum + cos/sin ----------------
            sin_t = st_pool.tile([P, NT, D], f16, tag="sin_t")
            cos_t = ct_pool.tile([P, NT, D], f16, tag="cos_t")
            for ti in reversed(range(NT)):
                ps = pmm.tile([P, D], f32, tag="pmm")
                for sj in range(NT - 1, ti - 1, -1):
                    nc.tensor.matmul(ps[:], (tril if sj == ti else onesb)[:],
                                     a_all[:, sj],
                                     start=(sj == NT - 1), stop=(sj == ti))
                u = sp_pool.tile([P, D], f32, tag="sp")
                w = sp_pool.tile([P, D], f32, tag="sp")
                nc.vector.add_range_wrap(u[:], ps[:], 0.0, PI, 2 * PI)
                nc.vector.add_range_wrap(w[:], ps[:], PI / 2, PI, 2 * PI)
                nc.scalar.activation(sin_t[:, ti], u[:], AF.Sin)
                nc.scalar.activation(cos_t[:, ti], w[:], AF.Sin)

            # transpose cos/sin to channel-major layout
            sin_c = sc_pool.tile([P, ND, T], f16, tag="sin_c")
            cos_c = cc_pool.tile([P, ND, T], f16, tag="cos_c")
            for src, dst in ((cos_t, cos_c), (sin_t, sin_c)):
                for dj in range(ND):
                    for th in range(2):
                        pst = ptr.tile([P, H], f16, tag="ptr")
                        for tq in range(4):
                            ti = th * 4 + tq
                            nc.tensor.transpose(pst[:, tq * P:(tq + 1) * P],
                                                src[:, ti, dj * P:(dj + 1) * P],
                                                ident[:])
                        nc.vector.tensor_copy(dst[:, dj, th * H:(th + 1) * H], pst[:])

            # ---------------- phase B: projections + rotation ----------------
            # v = x @ Wv'.T in [t, c] layout
            wv = w_pool.tile([P, NCH, C], f16, tag="w")
            for ci in range(NCH):
                nc.sync.dma_start(wv[:, ci], wv_d[ci])
            v_all = v_pool.tile([P, NT, C], f16, tag="v")
            for ti in range(NT):
                for ch in range(2):
                    ps = pmm.tile([P, H], f32, tag="pmm")
                    for ci in range(NCH):
                        nc.tensor.matmul(ps[:], x_all[:, ci, ti * P:(ti + 1) * P],
                                         wv[:, ci, ch * H:(ch + 1) * H],
                                         start=(ci == 0),
                                         stop=(ci == NCH - 1 and not has_bias))
                    if has_bias:
                        nc.tensor.matmul(ps[:], ones_row[:, :P],
                                         brows[:, 2, ch * H:(ch + 1) * H],
                                         start=False, stop=True)
                    nc.scalar.activation(v_all[:, ti, ch * H:(ch + 1) * H], ps[:], AF.Copy)
            # rotate v in place (t-major): pairs are column slabs [0:D], [D:C]
            for ti in range(NT):
                z0 = v_all[:, ti, 0:D]
                z1 = v_all[:, ti, D:C]
                ma = m_pool.tile([P, T], f16, tag="ma")
                mb = m_pool.tile([P, T], f16, tag="mb")
                mc = m_pool.tile([P, T], f16, tag="mc")
                nc.vector.tensor_mul(ma[:, 0:D], z0, cos_t[:, ti])
                nc.vector.tensor_mul(mb[:, 0:D], z1, sin_t[:, ti])
                nc.vector.tensor_mul(mc[:, 0:D], z0, sin_t[:, ti])
                nc.vector.tensor_sub(z0, ma[:, 0:D], mb[:, 0:D])
                nc.vector.tensor_mul(ma[:, D:C], z1, cos_t[:, ti])
                nc.vector.tensor_add(z1, mc[:, 0:D], ma[:, D:C])

            def proj_cmajor(w_dram, out_pool, tag, bias_idx):
                w_sb = w_pool.tile([P, NCH, C], f16, tag="w")
                for ci in range(NCH):
                    nc.sync.dma_start(w_sb[:, ci], w_dram[ci])
                out_all = out_pool.tile([P, NCH, T], f16, tag=tag)
                # Pair-ordered emission: compute co=p then co=p+4, rotating the
                # pair immediately so the DVE trails the PE instead of bunching
                # all rotations at the end of the projection.
                for p in range(ND):
                    for co in (p, p + 4):
                        for th in range(2):
                            ps = pmm.tile([P, H], f32, tag="pmm")
                            for ci in range(NCH):
                                nc.tensor.matmul(ps[:], w_sb[:, ci, co * P:(co + 1) * P],
                                                 x_all[:, ci, th * H:(th + 1) * H],
                                                 start=(ci == 0),
                                                 stop=(ci == NCH - 1 and not has_bias))
                            if has_bias:
                                nc.tensor.matmul(ps[:], brows[:, bias_idx, co * P:(co + 1) * P],
                                                 ones_row[:], start=False, stop=True)
                            nc.scalar.activation(out_all[:, co, th * H:(th + 1) * H],
                                                 ps[:], AF.Copy)
                    z0 = out_all[:, p, :]
                    z1 = out_all[:, p + 4, :]
                    cs = cos_c[:, p, :]
                    sn = sin_c[:, p, :]
                    ma = m_pool.tile([P, T], f16, tag="ma")
                    mb = m_pool.tile([P, T], f16, tag="mb")
                    mc = m_pool.tile([P, T], f16, tag="mc")
                    nc.vector.tensor_mul(ma[:], z0, cs)
                    nc.vector.tensor_mul(mb[:], z1, sn)
                    nc.vector.tensor_mul(mc[:], z0, sn)
                    nc.vector.tensor_sub(z0, ma[:], mb[:])
                    nc.vector.tensor_mul(ma[:], z1, cs)
                    nc.vector.tensor_add(z1, mc[:], ma[:])
                return out_all

            k_all = proj_cmajor(wk_d, k_pool, "k", 0)
            q_all = proj_cmajor(wq_d, q_pool, "q", 1)

            # ---------------- phase C: wei^T = softplus(q.k / sqrt(C)) -------
            # softplus(x) = ln(exp(x) + 1).  Exp and Ln live in different ACT
            # table sets on this compiler, and a table switch costs ~1.3us —
            # so run all Exps for a t-half, then all Lns (in place on the
            # fp16 spw tile), rather than alternating per block.
            spw = spw_pool.tile([P, NT, T], f16, tag="spw")
            for th in range(2):
                smax = 4 * th + 3
                for si in range(smax + 1):
                    ps = pmm.tile([P, H], f32, tag="pmm")
                    for ci in range(NCH):
                        nc.tensor.matmul(ps[:], q_all[:, ci, si * P:(si + 1) * P],
                                         k_all[:, ci, th * H:(th + 1) * H],
                                         start=(ci == 0), stop=(ci == NCH - 1))
                    nc.scalar.activation(spw[:, si, th * H:(th + 1) * H], ps[:],
                                         AF.Exp, scale=SCALE)
                for si in range(smax + 1):
                    nc.scalar.activation(spw[:, si, th * H:(th + 1) * H],
                                         spw[:, si, th * H:(th + 1) * H],
                                         AF.Ln, bias=1.0)
                for si in range(4 * th, 4 * th + 4):
                    off = si * P - th * H
                    if off > 0:
                        nc.vector.memset(spw[:, si, th * H:th * H + off], 0.0)
                    dcol = th * H + off
                    nc.vector.tensor_mul(spw[:, si, dcol:dcol + P],
                                         spw[:, si, dcol:dcol + P], triu[:])

            # ---------------- phase D: out^T = v.T @ wei^T, inverse-rotated --
            ro = xo_pool.tile([P, NCH, T], f16, tag="xo")
            for pp in range(ND):
                for th in range(2):
                    smax = 4 * th + 3
                    ps0 = pmm.tile([P, H], f32, tag="pmm")
                    ps1 = pmm.tile([P, H], f32, tag="pmm")
                    for sj in range(smax + 1):
                        nc.tensor.matmul(ps0[:], v_all[:, sj, pp * P:(pp + 1) * P],
                                         spw[:, sj, th * H:(th + 1) * H],
                                         start=(sj == 0), stop=(sj == smax))
                    for sj in range(smax + 1):
                        nc.tensor.matmul(ps1[:], v_all[:, sj, (pp + 4) * P:(pp + 5) * P],
                                         spw[:, sj, th * H:(th + 1) * H],
                                         start=(sj == 0), stop=(sj == smax))
                    cs = cos_c[:, pp, th * H:(th + 1) * H]
                    sn = sin_c[:, pp, th * H:(th + 1) * H]
                    # Evacuate PSUM via ACT first so the DVE rotation runs in
                    # 2x mode on all-fp16 SBUF operands (PSUM-sourced
                    # tensor_tensor is 1x and stalls the PE on PSUM slots).
                    oz = m_pool.tile([P, T], f16, tag="oz")
                    nc.scalar.activation(oz[:, 0:H], ps0[:], AF.Copy)
                    nc.scalar.activation(oz[:, H:T], ps1[:], AF.Copy)
                    ma = m_pool.tile([P, T], f16, tag="ma")
                    mb = m_pool.tile([P, T], f16, tag="mb")
                    # inv_rot: o0 = cos*z0 + sin*z1 ; o1 = cos*z1 - sin*z0
                    nc.vector.tensor_mul(ma[:, 0:H], oz[:, 0:H], cs)
                    nc.vector.tensor_mul(mb[:, 0:H], oz[:, H:T], sn)
                    nc.vector.tensor_add(ro[:, pp, th * H:(th + 1) * H],
                                         ma[:, 0:H], mb[:, 0:H])
                    nc.vector.tensor_mul(ma[:, H:T], oz[:, 0:H], sn)
                    nc.vector.tensor_mul(mb[:, H:T], oz[:, H:T], cs)
                    nc.vector.tensor_sub(ro[:, pp + 4, th * H:(th + 1) * H],
                                         mb[:, H:T], ma[:, H:T])

            # ---------------- phase E: y = ro.T @ Wp'.T ----------------------
            wp = w_pool.tile([P, NCH, C], f16, tag="w")
            for ci in range(NCH):
                nc.sync.dma_start(wp[:, ci], wp_d[ci])
            for ti in range(NT):
                for ch in range(2):
                    ps = pmm.tile([P, H], f32, tag="pmm")
                    for ci in range(NCH):
                        nc.tensor.matmul(ps[:], ro[:, ci, ti * P:(ti + 1) * P],
                                         wp[:, ci, ch * H:(ch + 1) * H],
                                         start=(ci == 0),
                                         stop=(ci == NCH - 1 and not has_bias))
                    if has_bias:
                        nc.tensor.matmul(ps[:], ones_row[:, :P],
                                         brows[:, 3, ch * H:(ch + 1) * H],
                                         start=False, stop=True)
                    yt = y_pool.tile([P, H], f32, tag="y")
                    nc.scalar.activation(yt[:], ps[:], AF.Copy)
                    nc.sync.dma_start(y_d[b, ti * P:(ti + 1) * P, ch * H:(ch + 1) * H],
                                      yt[:])

    nc.compile()
    return nc


def _get_program(has_bias: bool):
    key = ("prog", has_bias)
    if key not in _CACHE:
        _CACHE[key] = _build(has_bias)
    return _CACHE[key]


def _prep_host(x, idx, Wk, Wq, Wv, Wp, ang_emb, biases):
    perm = np.concatenate([np.arange(0, C, 2), np.arange(1, C, 2)])
    xT = np.ascontiguousarray(np.transpose(np.asarray(x, np.float32), (0, 2, 1)))
    xT16 = xT.astype(np.float16).reshape(NCORES, BPC, NCH, P, T)
    idx = np.asarray(idx).astype(np.int64)
    ang = np.asarray(ang_emb, np.float32)[idx]          # [B, T, D]
    ang16 = ang.astype(np.float16).reshape(NCORES, BPC, NT, P, D)

    def wtile(m):
        return np.ascontiguousarray(m).astype(np.float16).reshape(NCH, P, C)

    wkT = wtile(np.asarray(Wk, np.float32)[perm].T)
    wqT = wtile(np.asarray(Wq, np.float32)[perm].T)
    wvT = wtile(np.asarray(Wv, np.float32)[perm].T)
    wpT = wtile(np.asarray(Wp, np.float32)[:, perm].T)

    tril = np.tril(np.ones((P, P), np.float16))
    onesb = np.ones((P, P), np.float16)
    triu = np.triu(np.ones((P, P), np.float16))

    consts = dict(wkT=wkT, wqT=wqT, wvT=wvT, wpT=wpT,
                  tril=tril, onesb=onesb, triu=triu)
    bk, bq, bv, bp = (np.asarray(b_, np.float32) for b_ in biases)
    has_bias = any(np.any(b_ != 0) for b_ in (bk, bq, bv, bp))
    if has_bias:
        brows = np.stack([bk[perm], bq[perm], bv[perm], bp]).astype(np.float16)
        consts["biases"] = brows.reshape(4, 1, C)
    return xT16, ang16, consts, has_bias


def run_on_device(x, idx, Wk, Wq, Wv, Wp, ang_emb, biases, trace=False):
    _install_profile_hook()
    import concourse.bass_utils as bass_utils
    bass_utils.upload_artifacts = lambda tmpdir: "local://" + tmpdir
    from concourse.bass_utils import run_bass_kernel_spmd

    xT16, ang16, consts, has_bias = _prep_host(x, idx, Wk, Wq, Wv, Wp,
                                               ang_emb, biases)
    nc = _get_program(has_bias)
    in_maps = []
    for c in range(NCORES):
        m = {"xT": xT16[c], "ang": ang16[c]}
        m.update(consts)
        in_maps.append(m)
    res = run_bass_kernel_spmd(nc, in_maps, list(range(NCORES)), trace=trace)
    y = np.empty((B, T, C), np.float32)
    for c in range(NCORES):
        y[c * BPC:(c + 1) * BPC] = res.results[c]["y"]
    return y, res


def kernel(x, idx, Wk, bk, Wq, bq, Wv, bv, Wp, bp, ang_emb):
    y, _ = run_on_device(x, idx, Wk, Wq, Wv, Wp, ang_emb, (bk, bq, bv, bp))
    return y


# revision 7
# speedup vs baseline: 1.1168x; 1.0971x over previous
"""Trainium2 Bass kernel for CumsumAttention.

Full-input contract: kernel(**inputs) takes the complete (unsharded) inputs
and returns the full [B, T, C] float32 output. Internally the work is
data-parallel over the batch dimension across 8 NeuronCores (2 batches per
core); each core runs the same Bass/Tile program on its own batch shard.

Math (per batch, reference semantics):
  k = x @ Wk.T + bk ; q = x @ Wq.T + bq ; v = x @ Wv.T + bv
  angle[t] = sum_{s>=t} ang_emb[idx[s]]        (reverse cumsum over T)
  rot/inv_rot: per channel-pair rotation by angle
  wei  = softplus((rot(k) @ rot(q).T) / sqrt(C)) masked causally (s <= t)
  out  = inv_rot(wei @ rot(v)) @ Wp.T + bp

Device implementation notes:
  - Channel pairs (2i, 2i+1) are made contiguous by a host-side permutation
    baked into the weights, so the rotation operates on [0:C/2] / [C/2:C]
    slabs (elementwise, fp16 on DVE).
  - All matmul operands are fp16 (fp32 PSUM accumulation).  The reverse
    cumsum is a lower-triangular-ones matmul.  cos/sin come from the ACT
    Sin spline after an add_range_wrap range reduction; softplus is
    exp -> ln(1+x) (both in the natural_log_exp table set).
  - Attention is computed as wei^T (shape [s, t]) so both attention matmuls
    and the final projection consume operands in natural layouts; only
    cos/sin need an on-device transpose (PE transpose mode).
"""

import sys
import types
from contextlib import ExitStack

import numpy as np

if "/opt/trn_rl_repo" not in sys.path:
    sys.path.insert(0, "/opt/trn_rl_repo")

B, T, C = 16, 1024, 1024
D = C // 2
NCORES = 8
BPC = B // NCORES          # batches per core
P = 128                    # partitions
NT = T // P                # t tiles
NCH = C // P               # c tiles
ND = D // P                # d tiles (channel pairs)
H = 512                    # matmul free-dim block
PI = float(np.pi)
SCALE = float(C ** -0.5)

_CACHE = {}


def _install_profile_hook():
    """Register the axon NTFF profile hook if the image's antenv lacks it.

    Harmless when tracing is never requested; lets test harnesses pass
    trace=True to run_bass_kernel_spmd for HW timing.
    """
    try:
        import antenv
        from trn_agent_boot.trn_boot import _ntff_profile_via_ctypes
    except Exception:
        return
    if "antenv.axon_hooks" in sys.modules:
        return
    try:
        hook = _ntff_profile_via_ctypes("/opt/axon/libaxon_pjrt.so")
    except Exception:
        return
    mod = types.ModuleType("antenv.axon_hooks")
    mod.get_axon_ntff_profile_hook = lambda: hook
    mod.set_axon_ntff_profile_hook = lambda h: None
    sys.modules["antenv.axon_hooks"] = mod
    antenv.axon_hooks = mod


def _build(has_bias: bool):
    import concourse.bass as bass  # noqa: F401
    import concourse.mybir as mybir
    import concourse.tile as tile
    from concourse import bacc
    from concourse.masks import make_identity

    dt = mybir.dt
    AF = mybir.ActivationFunctionType
    OP = mybir.AluOpType
    f16 = dt.float16
    f32 = dt.float32

    # The ACT table-set chooser takes the first set containing each function,
    # which puts Exp and Ln in different sets and costs a ~1.3us table load
    # per switch.  Restrict Exp/Ln to the set that holds both (and Sin to
    # trig_and_small) so the program needs no mid-phase table switches.
    # Only membership is edited; dict order (= act_func_set_id) is preserved.
    import concourse.hw_specs as _hw_specs
    if not hasattr(_hw_specs, "_orig_get_activation_tables"):
        _hw_specs._orig_get_activation_tables = _hw_specs.get_activation_tables

        def _filtered_tables(arch):
            tabs = _hw_specs._orig_get_activation_tables(arch)
            for name, fns in tabs.items():
                if name != "natural_log_exp_and_others":
                    fns.discard(AF.Exp)
                    fns.discard(AF.Ln)
                if name != "trig_and_small":
                    fns.discard(AF.Sin)
            return tabs

        _hw_specs.get_activation_tables = _filtered_tables
        bacc.get_activation_tables = _filtered_tables

    nc = bacc.Bacc("TRN2", target_bir_lowering=False, debug=False,
                   num_devices=NCORES)

    xT_d = nc.dram_tensor("xT", [BPC, NCH, P, T], f16, kind="ExternalInput").ap()
    ang_d = nc.dram_tensor("ang", [BPC, NT, P, D], f16, kind="ExternalInput").ap()
    wk_d = nc.dram_tensor("wkT", [NCH, P, C], f16, kind="ExternalInput").ap()
    wq_d = nc.dram_tensor("wqT", [NCH, P, C], f16, kind="ExternalInput").ap()
    wv_d = nc.dram_tensor("wvT", [NCH, P, C], f16, kind="ExternalInput").ap()
    wp_d = nc.dram_tensor("wpT", [NCH, P, C], f16, kind="ExternalInput").ap()
    tril_d = nc.dram_tensor("tril", [P, P], f16, kind="ExternalInput").ap()
    ones_d = nc.dram_tensor("onesb", [P, P], f16, kind="ExternalInput").ap()
    triu_d = nc.dram_tensor("triu", [P, P], f16, kind="ExternalInput").ap()
    if has_bias:
        bias_d = nc.dram_tensor("biases", [4, 1, C], f16, kind="ExternalInput").ap()
    y_d = nc.dram_tensor("y", [BPC, T, C], f32, kind="ExternalOutput").ap()

    with tile.TileContext(nc) as tc, ExitStack() as ctx:
        const = ctx.enter_context(tc.tile_pool(name="const", bufs=1))
        w_pool = ctx.enter_context(tc.tile_pool(name="wpool", bufs=2))
        xo_pool = ctx.enter_context(tc.tile_pool(name="xopool", bufs=2))
        a_pool = ctx.enter_context(tc.tile_pool(name="apool", bufs=1))
        st_pool = ctx.enter_context(tc.tile_pool(name="stpool", bufs=1))
        ct_pool = ctx.enter_context(tc.tile_pool(name="ctpool", bufs=1))
        sc_pool = ctx.enter_context(tc.tile_pool(name="scpool", bufs=1))
        cc_pool = ctx.enter_context(tc.tile_pool(name="ccpool", bufs=1))
        k_pool = ctx.enter_context(tc.tile_pool(name="kpool", bufs=1))
        q_pool = ctx.enter_context(tc.tile_pool(name="qpool", bufs=1))
        v_pool = ctx.enter_context(tc.tile_pool(name="vpool", bufs=1))
        spw_pool = ctx.enter_context(tc.tile_pool(name="spwpool", bufs=1))
        m_pool = ctx.enter_context(tc.tile_pool(name="mpool", bufs=2))
        sp_pool = ctx.enter_context(tc.tile_pool(name="sppool", bufs=3))
        y_pool = ctx.enter_context(tc.tile_pool(name="ypool", bufs=2))
        pmm = ctx.enter_context(tc.tile_pool(name="pmm", bufs=6, space="PSUM"))
        ptr = ctx.enter_context(tc.tile_pool(name="ptr", bufs=2, space="PSUM"))

        ident = const.tile([P, P], f16)
        make_identity(nc, ident[:])
        tril = const.tile([P, P], f16)
        onesb = const.tile([P, P], f16)
        triu = const.tile([P, P], f16)
        nc.sync.dma_start(tril[:], tril_d[:])
        nc.sync.dma_start(onesb[:], ones_d[:])
        nc.sync.dma_start(triu[:], triu_d[:])
        if has_bias:
            ones_row = const.tile([1, H], f16)
            nc.gpsimd.memset(ones_row[:], 1.0)
            brows = const.tile([1, 4, C], f16)
            for i in range(4):
                nc.sync.dma_start(brows[:, i], bias_d[i])

        for b in range(BPC):
            # ---------------- input DMA ----------------
            # a tiles are loaded high-to-low: the reverse cumsum for t-tile
            # ti consumes s-tiles ti..NT-1, so ti=NT-1 is ready first.
            a_all = a_pool.tile([P, NT, D], f16, tag="a")
            for ti in reversed(range(NT)):
                nc.sync.dma_start(a_all[:, ti], ang_d[b, ti])
            x_all = xo_pool.tile([P, NCH, T], f16, tag="xo")
            for ci in range(NCH):
                nc.sync.dma_start(x_all[:, ci], xT_d[b, ci])

            # ---------------- phase A: angle cumsum + cos/sin ----------------
            sin_t = st_pool.tile([P, NT, D], f16, tag="sin_t")
            cos_t = ct_pool.tile([P, NT, D], f16, tag="cos_t")
            for ti in reversed(range(NT)):
                ps = pmm.tile([P, D], f32, tag="pmm")
                for sj in range(NT - 1, ti - 1, -1):
                    nc.tensor.matmul(ps[:], (tril if sj == ti else onesb)[:],
                                     a_all[:, sj],
                                     start=(sj == NT - 1), stop=(sj == ti))
                u = sp_pool.tile([P, D], f32, tag="sp")
                w = sp_pool.tile([P, D], f32, tag="sp")
                nc.vector.add_range_wrap(u[:], ps[:], 0.0, PI, 2 * PI)
                nc.vector.add_range_wrap(w[:], ps[:], PI / 2, PI, 2 * PI)
                nc.scalar.activation(sin_t[:, ti], u[:], AF.Sin)
                nc.scalar.activation(cos_t[:, ti], w[:], AF.Sin)

            # transpose cos/sin to channel-major layout
            sin_c = sc_pool.tile([P, ND, T], f16, tag="sin_c")
            cos_c = cc_pool.tile([P, ND, T], f16, tag="cos_c")
            for src, dst in ((cos_t, cos_c), (sin_t, sin_c)):
                for dj in range(ND):
                    for th in range(2):
                        pst = ptr.tile([P, H], f16, tag="ptr")
                        for tq in range(4):
                            ti = th * 4 + tq
                            nc.tensor.transpose(pst[:, tq * P:(tq + 1) * P],
                                                src[:, ti, dj * P:(dj + 1) * P],
                                                ident[:])
                        nc.vector.tensor_copy(dst[:, dj, th * H:(th + 1) * H], pst[:])

            # ---------------- phase B: projections + rotation ----------------
            # v = x @ Wv'.T in [t, c] layout
            wv = w_pool.tile([P, NCH, C], f16, tag="w")
            for ci in range(NCH):
                nc.sync.dma_start(wv[:, ci], wv_d[ci])
            v_all = v_pool.tile([P, NT, C], f16, tag="v")
            for ti in range(NT):
                for ch in range(2):
                    ps = pmm.tile([P, H], f32, tag="pmm")
                    for ci in range(NCH):
                        nc.tensor.matmul(ps[:], x_all[:, ci, ti * P:(ti + 1) * P],
                                         wv[:, ci, ch * H:(ch + 1) * H],
                                         start=(ci == 0),
                                         stop=(ci == NCH - 1 and not has_bias))
                    if has_bias:
                        nc.tensor.matmul(ps[:], ones_row[:, :P],
                                         brows[:, 2, ch * H:(ch + 1) * H],
                                         start=False, stop=True)
                    nc.scalar.activation(v_all[:, ti, ch * H:(ch + 1) * H], ps[:], AF.Copy)
            # rotate v in place (t-major): pairs are column slabs [0:D], [D:C]
            for ti in range(NT):
                z0 = v_all[:, ti, 0:D]
                z1 = v_all[:, ti, D:C]
                ma = m_pool.tile([P, T], f16, tag="ma")
                mb = m_pool.tile([P, T], f16, tag="mb")
                mc = m_pool.tile([P, T], f16, tag="mc")
                nc.vector.tensor_mul(ma[:, 0:D], z0, cos_t[:, ti])
                nc.vector.tensor_mul(mb[:, 0:D], z1, sin_t[:, ti])
                nc.vector.tensor_mul(mc[:, 0:D], z0, sin_t[:, ti])
                nc.vector.tensor_sub(z0, ma[:, 0:D], mb[:, 0:D])
                nc.vector.tensor_mul(ma[:, D:C], z1, cos_t[:, ti])
                nc.vector.tensor_add(z1, mc[:, 0:D], ma[:, D:C])

            def proj_cmajor(w_dram, out_pool, tag, bias_idx):
                w_sb = w_pool.tile([P, NCH, C], f16, tag="w")
                for ci in range(NCH):
                    nc.sync.dma_start(w_sb[:, ci], w_dram[ci])
                out_all = out_pool.tile([P, NCH, T], f16, tag=tag)
                # Pair-ordered emission: compute co=p then co=p+4, rotating the
                # pair immediately so the DVE trails the PE instead of bunching
                # all rotations at the end of the projection.
                for p in range(ND):
                    for co in (p, p + 4):
                        for th in range(2):
                            ps = pmm.tile([P, H], f32, tag="pmm")
                            for ci in range(NCH):
                                nc.tensor.matmul(ps[:], w_sb[:, ci, co * P:(co + 1) * P],
                                                 x_all[:, ci, th * H:(th + 1) * H],
                                                 start=(ci == 0),
                                                 stop=(ci == NCH - 1 and not has_bias))
                            if has_bias:
                                nc.tensor.matmul(ps[:], brows[:, bias_idx, co * P:(co + 1) * P],
                                                 ones_row[:], start=False, stop=True)
                            nc.scalar.activation(out_all[:, co, th * H:(th + 1) * H],
                                                 ps[:], AF.Copy)
                    z0 = out_all[:, p, :]
                    z1 = out_all[:, p + 4, :]
                    cs = cos_c[:, p, :]
                    sn = sin_c[:, p, :]
                    ma = m_pool.tile([P, T], f16, tag="ma")
                    mb = m_pool.tile([P, T], f16, tag="mb")
                    mc = m_pool.tile([P, T], f16, tag="mc")
                    nc.vector.tensor_mul(ma[:], z0, cs)
                    nc.vector.tensor_mul(mb[:], z1, sn)
                    nc.vector.tensor_mul(mc[:], z0, sn)
                    nc.vector.tensor_sub(z0, ma[:], mb[:])
                    nc.vector.tensor_mul(ma[:], z1, cs)
                    nc.vector.tensor_add(z1, mc[:], ma[:])
                return out_all

            k_all = proj_cmajor(wk_d, k_pool, "k", 0)
            q_all = proj_cmajor(wq_d, q_pool, "q", 1)

            # ---------------- phase C: wei^T = softplus(q.k / sqrt(C)) -------
            # softplus(x) = ln(exp(x) + 1).  Exp and Ln live in different ACT
            # table sets on this compiler, and a table switch costs ~1.3us —
            # so run all Exps for a t-half, then all Lns (in place on the
            # fp16 spw tile), rather than alternating per block.
            spw = spw_pool.tile([P, NT, T], f16, tag="spw")
            for th in range(2):
                smax = 4 * th + 3
                for si in range(smax + 1):
                    ps = pmm.tile([P, H], f32, tag="pmm")
                    for ci in range(NCH):
                        nc.tensor.matmul(ps[:], q_all[:, ci, si * P:(si + 1) * P],
                                         k_all[:, ci, th * H:(th + 1) * H],
                                         start=(ci == 0), stop=(ci == NCH - 1))
                    nc.scalar.activation(spw[:, si, th * H:(th + 1) * H], ps[:],
                                         AF.Exp, scale=SCALE)
                for si in range(smax + 1):
                    nc.scalar.activation(spw[:, si, th * H:(th + 1) * H],
                                         spw[:, si, th * H:(th + 1) * H],
                                         AF.Ln, bias=1.0)
                for si in range(4 * th, 4 * th + 4):
                    off = si * P - th * H
                    if off > 0:
                        nc.vector.memset(spw[:, si, th * H:th * H + off], 0.0)
                    dcol = th * H + off
                    nc.vector.tensor_mul(spw[:, si, dcol:dcol + P],
                                         spw[:, si, dcol:dcol + P], triu[:])

            # ---------------- phase D: out^T = v.T @ wei^T, inverse-rotated --
            ro = xo_pool.tile([P, NCH, T], f16, tag="xo")
            for pp in range(ND):
                for th in range(2):
                    smax = 4 * th + 3
                    ps0 = pmm.tile([P, H], f32, tag="pmm")
                    ps1 = pmm.tile([P, H], f32, tag="pmm")
                    for sj in range(smax + 1):
                        nc.tensor.matmul(ps0[:], v_all[:, sj, pp * P:(pp + 1) * P],
                                         spw[:, sj, th * H:(th + 1) * H],
                                         start=(sj == 0), stop=(sj == smax))
                    for sj in range(smax + 1):
                        nc.tensor.matmul(ps1[:], v_all[:, sj, (pp + 4) * P:(pp + 5) * P],
                                         spw[:, sj, th * H:(th + 1) * H],
                                         start=(sj == 0), stop=(sj == smax))
                    cs = cos_c[:, pp, th * H:(th + 1) * H]
                    sn = sin_c[:, pp, th * H:(th + 1) * H]
                    # Evacuate PSUM via ACT first so the DVE rotation runs in
                    # 2x mode on all-fp16 SBUF operands (PSUM-sourced
                    # tensor_tensor is 1x and stalls the PE on PSUM slots).
                    oz = m_pool.tile([P, T], f16, tag="oz")
                    nc.scalar.activation(oz[:, 0:H], ps0[:], AF.Copy)
                    nc.scalar.activation(oz[:, H:T], ps1[:], AF.Copy)
                    ma = m_pool.tile([P, T], f16, tag="ma")
                    mb = m_pool.tile([P, T], f16, tag="mb")
                    # inv_rot: o0 = cos*z0 + sin*z1 ; o1 = cos*z1 - sin*z0
                    nc.vector.tensor_mul(ma[:, 0:H], oz[:, 0:H], cs)
                    nc.vector.tensor_mul(mb[:, 0:H], oz[:, H:T], sn)
                    nc.vector.tensor_add(ro[:, pp, th * H:(th + 1) * H],
                                         ma[:, 0:H], mb[:, 0:H])
                    nc.vector.tensor_mul(ma[:, H:T], oz[:, 0:H], sn)
                    nc.vector.tensor_mul(mb[:, H:T], oz[:, H:T], cs)
                    nc.vector.tensor_sub(ro[:, pp + 4, th * H:(th + 1) * H],
                                         mb[:, H:T], ma[:, H:T])

            # ---------------- phase E: y = ro.T @ Wp'.T ----------------------
            wp = w_pool.tile([P, NCH, C], f16, tag="w")
            for ci in range(NCH):
                nc.sync.dma_start(wp[:, ci], wp_d[ci])
            for ti in range(NT):
                for ch in range(2):
                    ps = pmm.tile([P, H], f32, tag="pmm")
                    for ci in range(NCH):
                        nc.tensor.matmul(ps[:], ro[:, ci, ti * P:(ti + 1) * P],
                                         wp[:, ci, ch * H:(ch + 1) * H],
                                         start=(ci == 0),
                                         stop=(ci == NCH - 1 and not has_bias))
                    if has_bias:
                        nc.tensor.matmul(ps[:], ones_row[:, :P],
                                         brows[:, 3, ch * H:(ch + 1) * H],
                                         start=False, stop=True)
                    yt = y_pool.tile([P, H], f32, tag="y")
                    nc.scalar.activation(yt[:], ps[:], AF.Copy)
                    nc.sync.dma_start(y_d[b, ti * P:(ti + 1) * P, ch * H:(ch + 1) * H],
                                      yt[:])

    nc.compile()
    return nc


def _get_program(has_bias: bool):
    key = ("prog", has_bias)
    if key not in _CACHE:
        _CACHE[key] = _build(has_bias)
    return _CACHE[key]


def _prep_host(x, idx, Wk, Wq, Wv, Wp, ang_emb, biases):
    perm = np.concatenate([np.arange(0, C, 2), np.arange(1, C, 2)])
    xT = np.ascontiguousarray(np.transpose(np.asarray(x, np.float32), (0, 2, 1)))
    xT16 = xT.astype(np.float16).reshape(NCORES, BPC, NCH, P, T)
    idx = np.asarray(idx).astype(np.int64)
    ang = np.asarray(ang_emb, np.float32)[idx]          # [B, T, D]
    ang16 = ang.astype(np.float16).reshape(NCORES, BPC, NT, P, D)

    def wtile(m):
        return np.ascontiguousarray(m).astype(np.float16).reshape(NCH, P, C)

    wkT = wtile(np.asarray(Wk, np.float32)[perm].T)
    wqT = wtile(np.asarray(Wq, np.float32)[perm].T)
    wvT = wtile(np.asarray(Wv, np.float32)[perm].T)
    wpT = wtile(np.asarray(Wp, np.float32)[:, perm].T)

    tril = np.tril(np.ones((P, P), np.float16))
    onesb = np.ones((P, P), np.float16)
    triu = np.triu(np.ones((P, P), np.float16))

    consts = dict(wkT=wkT, wqT=wqT, wvT=wvT, wpT=wpT,
                  tril=tril, onesb=onesb, triu=triu)
    bk, bq, bv, bp = (np.asarray(b_, np.float32) for b_ in biases)
    has_bias = any(np.any(b_ != 0) for b_ in (bk, bq, bv, bp))
    if has_bias:
        brows = np.stack([bk[perm], bq[perm], bv[perm], bp]).astype(np.float16)
        consts["biases"] = brows.reshape(4, 1, C)
    return xT16, ang16, consts, has_bias


def run_on_device(x, idx, Wk, Wq, Wv, Wp, ang_emb, biases, trace=False):
    _install_profile_hook()
    import concourse.bass_utils as bass_utils
    bass_utils.upload_artifacts = lambda tmpdir: "local://" + tmpdir
    from concourse.bass_utils import run_bass_kernel_spmd

    xT16, ang16, consts, has_bias = _prep_host(x, idx, Wk, Wq, Wv, Wp,
                                               ang_emb, biases)
    nc = _get_program(has_bias)
    in_maps = []
    for c in range(NCORES):
        m = {"xT": xT16[c], "ang": ang16[c]}
        m.update(consts)
        in_maps.append(m)
    res = run_bass_kernel_spmd(nc, in_maps, list(range(NCORES)), trace=trace)
    y = np.empty((B, T, C), np.float32)
    for c in range(NCORES):
        y[c * BPC:(c + 1) * BPC] = res.results[c]["y"]
    return y, res


def kernel(x, idx, Wk, bk, Wq, bq, Wv, bv, Wp, bp, ang_emb):
    y, _ = run_on_device(x, idx, Wk, Wq, Wv, Wp, ang_emb, (bk, bq, bv, bp))
    return y
